# revision 6
# baseline (speedup 1.0000x reference)
"""Multi-head causal self-attention (B=2, S=2048, D=1024, H=16) on 8 TRN2 cores.

Sharding: core = b*4 + hg  (b in {0,1} batch, hg in {0..3} head-group of 4 heads).
Per core: project qT/kT (pair-packed [128, S], bf16) and v ([S, 64] blocks, bf16),
compute transposed scores S^T = K Q^T per head (k on partitions), causal mask
added in PSUM via identity-matmul, exp on ScalarE (bf16 out), PV matmul with a
ones-column appended to V so row 64 of the accumulator is the softmax sum,
normalization via reciprocal + DMA partition-broadcast + tensor mul, then the
partial output projection. Host sums the 4 per-batch partials and adds
(b_v @ w_o.T + b_o); b_k is dropped (softmax is invariant to per-query
constants); b_q is applied on-device. Matmul operands are bf16 (fp32 moving
operand streams at half rate on TRN2); all accumulation is fp32 in PSUM.
"""

import numpy as np
import ml_dtypes

import concourse.bass as bass
import concourse.mybir as mybir
import concourse.tile as tile
from concourse import bacc
from concourse.bass_utils import run_bass_kernel_spmd

B, S, D, H, DK = 2, 2048, 1024, 16, 64
N_CORES = 8
F32 = mybir.dt.float32
BF16 = mybir.dt.bfloat16
NPBF = ml_dtypes.bfloat16
AF = mybir.ActivationFunctionType
NEG_BIG = -1.0e9


def _build(debug=False):
    nc = bacc.Bacc("TRN2", target_bir_lowering=False, debug=False,
                   num_devices=N_CORES)
    xT = nc.dram_tensor("xT", [D, S], BF16, kind="ExternalInput").ap()
    wqT = nc.dram_tensor("wqT", [D, 256], BF16, kind="ExternalInput").ap()
    wkT = nc.dram_tensor("wkT", [D, 256], BF16, kind="ExternalInput").ap()
    wvT = nc.dram_tensor("wvT", [D, 256], BF16, kind="ExternalInput").ap()
    woT = nc.dram_tensor("woT", [256, D], BF16, kind="ExternalInput").ap()
    bq2 = nc.dram_tensor("bq2", [128, 2], F32, kind="ExternalInput").ap()
    masks = nc.dram_tensor("masks", [128, 2048], BF16, kind="ExternalInput").ap()
    ident = nc.dram_tensor("ident", [128, 128], BF16, kind="ExternalInput").ap()
    y = nc.dram_tensor("y", [S, D], F32, kind="ExternalOutput").ap()
    dbg = {}
    if debug:
        for nm, shp in [("qT", [128, 2, S]), ("kT", [128, 2, S]),
                        ("vv", [128, 16, 260]), ("oT", [128, 2, S])]:
            dbg[nm] = nc.dram_tensor(nm, shp, BF16, kind="ExternalOutput").ap()

    NQC = 4          # q-chunks of 512
    QC = 512
    NKT = S // 128   # k tiles

    with tile.TileContext(nc) as tc, \
            nc.allow_low_precision(reason="bf16 attention kernel"):
        with (
            tc.tile_pool(name="persist", bufs=1) as persist,
            tc.tile_pool(name="kqv", bufs=2) as kqv,
        ):
            qT_sb = [kqv.tile([128, S], BF16, tag="qT", name=f"qT{p}") for p in range(2)]
            kT_sb = [kqv.tile([128, S], BF16, tag="kT", name=f"kT{p}") for p in range(2)]
            v_sb = [persist.tile([128, 4 * 65], BF16, tag=f"v{t}", name=f"v{t}")
                    for t in range(NKT)]
            outT_sb = [persist.tile([128, S], BF16, tag=f"oT{p}", name=f"oTs{p}")
                       for p in range(2)]
            wo_sb = [persist.tile([128, D], BF16, tag=f"wo{p}", name=f"wo{p}")
                     for p in range(2)]
            mask_sb = persist.tile([128, 2048], BF16, tag="masks")
            id_sb = persist.tile([128, 128], BF16, tag="ident")
            bq_sb = persist.tile([128, 2], F32, tag="bq")

            nc.sync.dma_start(out=mask_sb, in_=masks)
            nc.sync.dma_start(out=id_sb, in_=ident)
            nc.sync.dma_start(out=bq_sb, in_=bq2)
            for p in range(2):
                nc.sync.dma_start(out=wo_sb[p], in_=woT[p * 128:(p + 1) * 128, :])

            # ---------------- Phase A: projections ----------------
            with (
                tc.tile_pool(name="xw", bufs=1) as xw,
                tc.tile_pool(name="pps", bufs=4, space="PSUM") as pps,
            ):
                xt = [xw.tile([128, S], BF16, tag=f"x{c}", name=f"xt{c}") for c in range(8)]
                wq_sb = [xw.tile([128, 256], BF16, tag=f"wq{c}", name=f"wqs{c}") for c in range(8)]
                wk_sb = [xw.tile([128, 256], BF16, tag=f"wk{c}", name=f"wks{c}") for c in range(8)]
                wv_sb = [xw.tile([128, 256], BF16, tag=f"wv{c}", name=f"wvs{c}") for c in range(8)]
                for c in range(8):
                    nc.sync.dma_start(out=xt[c], in_=xT[c * 128:(c + 1) * 128, :])
                    nc.sync.dma_start(out=wq_sb[c], in_=wqT[c * 128:(c + 1) * 128, :])
                    nc.sync.dma_start(out=wk_sb[c], in_=wkT[c * 128:(c + 1) * 128, :])
                    nc.sync.dma_start(out=wv_sb[c], in_=wvT[c * 128:(c + 1) * 128, :])

                for p in range(2):
                    for j in range(NQC):
                        ps_q = pps.tile([128, QC], F32, tag="proj", name="ps_q")
                        for c in range(8):
                            nc.tensor.matmul(
                                ps_q, wq_sb[c][:, p * 128:(p + 1) * 128],
                                xt[c][:, j * QC:(j + 1) * QC],
                                start=(c == 0), stop=(c == 7))
                        nc.vector.tensor_scalar_add(
                            qT_sb[p][:, j * QC:(j + 1) * QC], ps_q,
                            bq_sb[:, p:p + 1])
                        ps_k = pps.tile([128, QC], F32, tag="proj", name="ps_k")
                        for c in range(8):
                            nc.tensor.matmul(
                                ps_k, wk_sb[c][:, p * 128:(p + 1) * 128],
                                xt[c][:, j * QC:(j + 1) * QC],
                                start=(c == 0), stop=(c == 7))
                        nc.vector.tensor_copy(
                            kT_sb[p][:, j * QC:(j + 1) * QC], ps_k)

                for t in range(NKT):
                    ps_v = pps.tile([128, 256], F32, tag="vps", name="ps_v")
                    for c in range(8):
                        nc.tensor.matmul(
                            ps_v, xt[c][:, t * 128:(t + 1) * 128], wv_sb[c],
                            start=(c == 0), stop=(c == 7))
                    v_view = v_sb[t].rearrange("p (h w) -> p h w", w=65)
                    nc.vector.memset(v_view[:, :, 64:65], 1.0)
                    nc.vector.tensor_copy(
                        v_view[:, :, 0:64],
                        ps_v.rearrange("p (h w) -> p h w", w=64))

            if debug:
                for p in range(2):
                    nc.sync.dma_start(out=dbg["qT"][:, p, :], in_=qT_sb[p])
                    nc.sync.dma_start(out=dbg["kT"][:, p, :], in_=kT_sb[p])
                for t in range(NKT):
                    nc.sync.dma_start(out=dbg["vv"][:, t, :], in_=v_sb[t])

            # ---------------- Phase B: attention ----------------
            with (
                tc.tile_pool(name="sq", bufs=2, space="PSUM") as sqp,
                tc.tile_pool(name="ops", bufs=2, space="PSUM") as opp,
                tc.tile_pool(name="ep", bufs=3) as ep,
                tc.tile_pool(name="osb", bufs=3) as osbp,
                tc.tile_pool(name="rp", bufs=3) as rp,
            ):
                deferred_norm = [None]

                def emit_norm(p_, q0_, o_ps_):
                    for s in range(2):
                        o_sb = osbp.tile([65, QC], F32, tag="osb", name="o_sb")
                        nc.vector.tensor_copy(o_sb, o_ps_[s])
                        recip = rp.tile([1, QC], BF16, tag="recip", name="recip")
                        nc.vector.reciprocal(recip, o_sb[64:65, :])
                        bc = rp.tile([64, QC], BF16, tag="bc", name="bc")
                        nc.gpsimd.partition_broadcast(bc, recip)
                        nc.vector.tensor_mul(
                            outT_sb[p_][s * 64:(s + 1) * 64, q0_:q0_ + QC],
                            o_sb[0:64, :], bc)

                for p in range(2):
                    for qc in range(NQC):
                        q0 = qc * QC
                        nkt = 4 * qc + 4
                        o_ps = [opp.tile([65, QC], F32, tag=f"o{s}", name=f"ops{s}")
                                for s in range(2)]
                        pend = None
                        for kt in range(nkt):
                            o = kt * 128 - q0
                            diag = o >= 0
                            s_ab = sqp.tile([128, 2 * QC], F32, tag="sq", name="s_ab")
                            for s in range(2):
                                half = s * QC
                                lo = o if diag else 0
                                if diag:
                                    oi = o // 128
                                    nc.tensor.matmul(
                                        s_ab[:, half:half + o + 128], id_sb,
                                        mask_sb[:, oi * 512:oi * 512 + o + 128],
                                        start=True, stop=False,
                                        skip_group_check=True)
                                nc.tensor.matmul(
                                    s_ab[:, half + lo:half + QC],
                                    kT_sb[p][s * 64:(s + 1) * 64,
                                             kt * 128:(kt + 1) * 128],
                                    qT_sb[p][s * 64:(s + 1) * 64,
                                             q0 + lo:q0 + QC],
                                    start=not diag, stop=True,
                                    tile_position=(s * 64, 0),
                                    skip_group_check=True)
                            e_ab = ep.tile([128, 2 * QC], BF16, tag="e", name="e_ab")
                            nc.scalar.activation(e_ab, s_ab, AF.Exp, scale=0.125)
                            if kt == 2 and deferred_norm[0] is not None:
                                deferred_norm[0]()
                                deferred_norm[0] = None
                            if pend is not None:
                                _kt, _e, _lo = pend
                                for s in range(2):
                                    hb = 2 * p + s
                                    nc.tensor.matmul(
                                        o_ps[s][:, _lo:QC],
                                        v_sb[_kt][:, hb * 65:(hb + 1) * 65],
                                        _e[:, s * QC + _lo:(s + 1) * QC],
                                        start=(_kt == 0), stop=False,
                                        skip_group_check=True)
                            pend = (kt, e_ab, o if diag else 0)
                        _kt, _e, _lo = pend
                        for s in range(2):
                            hb = 2 * p + s
                            nc.tensor.matmul(
                                o_ps[s][:, _lo:QC],
                                v_sb[_kt][:, hb * 65:(hb + 1) * 65],
                                _e[:, s * QC + _lo:(s + 1) * QC],
                                start=False, stop=True,
                                skip_group_check=True)
                        deferred_norm[0] = (
                            lambda p_=p, q0_=q0, o_=o_ps: emit_norm(p_, q0_, o_))
                if deferred_norm[0] is not None:
                    deferred_norm[0]()
                    deferred_norm[0] = None

            if debug:
                for p in range(2):
                    nc.sync.dma_start(out=dbg["oT"][:, p, :], in_=outT_sb[p])

            # ---------------- Phase C: output projection ----------------
            with (
                tc.tile_pool(name="fps", bufs=4, space="PSUM") as fps,
                tc.tile_pool(name="fsb", bufs=4) as fsb,
            ):
                for qt in range(16):
                    for oc in range(2):
                        f_ps = fps.tile([128, 512], F32, tag="f", name="f_ps")
                        for p in range(2):
                            nc.tensor.matmul(
                                f_ps, outT_sb[p][:, qt * 128:(qt + 1) * 128],
                                wo_sb[p][:, oc * 512:(oc + 1) * 512],
                                start=(p == 0), stop=(p == 1))
                        f_sb = fsb.tile([128, 512], F32, tag="f", name="f_sb")
                        if (qt * 2 + oc) % 2 == 0:
                            nc.scalar.copy(f_sb, f_ps)
                        else:
                            nc.vector.tensor_copy(f_sb, f_ps)
                        nc.sync.dma_start(
                            out=y[qt * 128:(qt + 1) * 128,
                                  oc * 512:(oc + 1) * 512],
                            in_=f_sb)

    nc.compile()
    return nc


_cached = {}


def _get_nc(debug=False):
    key = bool(debug)
    if key not in _cached:
        _cached[key] = _build(debug)
    return _cached[key]


def _host_masks():
    k = np.arange(128)[:, None]
    m = np.zeros((128, 4, 512), np.float32)
    for oi in range(4):
        o = oi * 128
        q = np.arange(512)[None, :]
        m[:, oi, :] = np.where(q >= k + o, 0.0, NEG_BIG)
    return m.reshape(128, 2048).astype(NPBF)


def _prep_inputs(x, w_q, b_q, w_k, w_v):
    masks = _host_masks()
    ident = np.eye(128, dtype=NPBF)
    wqT_f = np.ascontiguousarray(w_q.T).astype(NPBF)
    wkT_f = np.ascontiguousarray(w_k.T).astype(NPBF)
    wvT_f = np.ascontiguousarray(w_v.T).astype(NPBF)
    in_maps = []
    for core in range(N_CORES):
        b, hg = divmod(core, 4)
        cs = slice(hg * 256, (hg + 1) * 256)
        in_maps.append({
            "xT": np.ascontiguousarray(x[b].T).astype(NPBF),
            "wqT": np.ascontiguousarray(wqT_f[:, cs]),
            "wkT": np.ascontiguousarray(wkT_f[:, cs]),
            "wvT": np.ascontiguousarray(wvT_f[:, cs]),
            "bq2": np.ascontiguousarray(
                b_q[hg * 256:(hg + 1) * 256].reshape(2, 128).T.astype(np.float32)),
            "masks": masks,
            "ident": ident,
        })
    return in_maps


def _numpy_reference(x, attention_mask, w_q, b_q, w_k, b_k, w_v, b_v, w_o, b_o):
    x = x.astype(np.float64)
    q = (x @ w_q.T + b_q).reshape(B, S, H, DK).transpose(0, 2, 1, 3)
    k = (x @ w_k.T + b_k).reshape(B, S, H, DK).transpose(0, 2, 1, 3)
    v = (x @ w_v.T + b_v).reshape(B, S, H, DK).transpose(0, 2, 1, 3)
    scores = np.einsum("bhqd,bhkd->bhqk", q, k) / np.sqrt(DK)
    causal = np.tril(np.ones((S, S), bool))
    mask = causal[None, None] & (attention_mask[:, None, None, :] != 0)
    scores = np.where(mask, scores, -np.inf)
    scores -= scores.max(-1, keepdims=True)
    e = np.exp(scores)
    attn = e / e.sum(-1, keepdims=True)
    out = np.einsum("bhqk,bhkd->bhqd", attn, v)
    out = out.transpose(0, 2, 1, 3).reshape(B, S, D)
    return (out @ w_o.T + b_o).astype(np.float32)


def kernel(x, attention_mask, w_q, b_q, w_k, b_k, w_v, b_v, w_o, b_o,
           _debug=False, _trace=False):
    x = np.asarray(x, np.float32)
    attention_mask = np.asarray(attention_mask)
    if not np.all(attention_mask != 0):
        return _numpy_reference(np.asarray(x), np.asarray(attention_mask),
                                *[np.asarray(a) for a in
                                  (w_q, b_q, w_k, b_k, w_v, b_v, w_o, b_o)])
    w_q, w_k, w_v, w_o = [np.asarray(w, np.float32) for w in (w_q, w_k, w_v, w_o)]
    b_q, b_k, b_v, b_o = [np.asarray(b, np.float32) for b in (b_q, b_k, b_v, b_o)]

    nc = _get_nc(_debug)
    in_maps = _prep_inputs(x, w_q, b_q, w_k, w_v)
    woT_f = np.ascontiguousarray(w_o.T).astype(NPBF)
    for core in range(N_CORES):
        hg = core % 4
        in_maps[core]["woT"] = np.ascontiguousarray(
            woT_f[hg * 256:(hg + 1) * 256, :])

    res = run_bass_kernel_spmd(nc, in_maps, list(range(N_CORES)), trace=_trace)
    const_row = (b_v @ w_o.T + b_o).astype(np.float32)
    y = np.zeros((B, S, D), np.float32)
    for core in range(N_CORES):
        b = core // 4
        y[b] += res.results[core]["y"]
    y += const_row
    if _debug or _trace:
        return y, res
    return y


# revision 9
# speedup vs baseline: 1.0635x; 1.0635x over previous
"""Multi-head causal self-attention (B=2, S=2048, D=1024, H=16) on 8 TRN2 cores.

Sharding: core = b*4 + hg  (b in {0,1} batch, hg in {0..3} head-group of 4 heads).
Per core: project qT/kT (pair-packed [128, S], bf16) and v ([S, 64] blocks, bf16),
compute transposed scores S^T = K Q^T per head (k on partitions), causal mask
added in PSUM via identity-matmul, exp on ScalarE (bf16 out), PV matmul with a
ones-column appended to V so row 64 of the accumulator is the softmax sum,
normalization via reciprocal + DMA partition-broadcast + tensor mul, then the
partial output projection. Host sums the 4 per-batch partials and adds
(b_v @ w_o.T + b_o); b_k is dropped (softmax is invariant to per-query
constants); b_q is applied on-device. Matmul operands are bf16 (fp32 moving
operand streams at half rate on TRN2); all accumulation is fp32 in PSUM.
"""

import numpy as np
import ml_dtypes

import concourse.bass as bass
import concourse.mybir as mybir
import concourse.tile as tile
from concourse import bacc
from concourse.bass_utils import run_bass_kernel_spmd

B, S, D, H, DK = 2, 2048, 1024, 16, 64
N_CORES = 8
F32 = mybir.dt.float32
BF16 = mybir.dt.bfloat16
NPBF = ml_dtypes.bfloat16
AF = mybir.ActivationFunctionType
NEG_BIG = -1.0e9


def _build(debug=False):
    nc = bacc.Bacc("TRN2", target_bir_lowering=False, debug=False,
                   num_devices=N_CORES)
    xT = nc.dram_tensor("xT", [D, S], BF16, kind="ExternalInput").ap()
    wqT = nc.dram_tensor("wqT", [D, 256], BF16, kind="ExternalInput").ap()
    wkT = nc.dram_tensor("wkT", [D, 256], BF16, kind="ExternalInput").ap()
    wvT = nc.dram_tensor("wvT", [D, 256], BF16, kind="ExternalInput").ap()
    woT = nc.dram_tensor("woT", [256, D], BF16, kind="ExternalInput").ap()
    bq2 = nc.dram_tensor("bq2", [128, 2], F32, kind="ExternalInput").ap()
    masks = nc.dram_tensor("masks", [128, 2048], BF16, kind="ExternalInput").ap()
    ident = nc.dram_tensor("ident", [128, 128], BF16, kind="ExternalInput").ap()
    y = nc.dram_tensor("y", [S, D], F32, kind="ExternalOutput").ap()
    dbg = {}
    if debug:
        for nm, shp in [("qT", [128, 2, S]), ("kT", [128, 2, S]),
                        ("vv", [128, 16, 260]), ("oT", [128, 2, S])]:
            dbg[nm] = nc.dram_tensor(nm, shp, BF16, kind="ExternalOutput").ap()

    NQC = 4          # q-chunks of 512
    QC = 512
    NKT = S // 128   # k tiles

    with tile.TileContext(nc) as tc, \
            nc.allow_low_precision(reason="bf16 attention kernel"):
        with (
            tc.tile_pool(name="persist", bufs=1) as persist,
            tc.tile_pool(name="kqv", bufs=2) as kqv,
        ):
            qT_sb = [kqv.tile([128, S], BF16, tag="qT", name=f"qT{p}") for p in range(2)]
            kT_sb = [kqv.tile([128, S], BF16, tag="kT", name=f"kT{p}") for p in range(2)]
            v_sb = [persist.tile([128, 4 * 65], BF16, tag=f"v{t}", name=f"v{t}")
                    for t in range(NKT)]
            outT_sb = [persist.tile([128, S], BF16, tag=f"oT{p}", name=f"oTs{p}")
                       for p in range(2)]
            wo_sb = [persist.tile([128, D], BF16, tag=f"wo{p}", name=f"wo{p}")
                     for p in range(2)]
            mask_sb = persist.tile([128, 2048], BF16, tag="masks")
            id_sb = persist.tile([128, 128], BF16, tag="ident")
            bq_sb = persist.tile([128, 2], F32, tag="bq")

            # ---------------- Phase A: projections ----------------
            with (
                tc.tile_pool(name="xw", bufs=1) as xw,
                tc.tile_pool(name="pps", bufs=4, space="PSUM") as pps,
            ):
                xt = [xw.tile([128, S], BF16, tag=f"x{c}", name=f"xt{c}") for c in range(8)]
                wq_sb = [xw.tile([128, 256], BF16, tag=f"wq{c}", name=f"wqs{c}") for c in range(8)]
                wk_sb = [xw.tile([128, 256], BF16, tag=f"wk{c}", name=f"wks{c}") for c in range(8)]
                wv_sb = [xw.tile([128, 256], BF16, tag=f"wv{c}", name=f"wvs{c}") for c in range(8)]
                for c in range(8):
                    nc.sync.dma_start(out=xt[c], in_=xT[c * 128:(c + 1) * 128, :])
                for c in range(8):
                    nc.sync.dma_start(out=wq_sb[c], in_=wqT[c * 128:(c + 1) * 128, :])
                    nc.sync.dma_start(out=wk_sb[c], in_=wkT[c * 128:(c + 1) * 128, :])
                for c in range(8):
                    nc.sync.dma_start(out=wv_sb[c], in_=wvT[c * 128:(c + 1) * 128, :])
                nc.sync.dma_start(out=bq_sb, in_=bq2)
                nc.sync.dma_start(out=mask_sb, in_=masks)
                nc.sync.dma_start(out=id_sb, in_=ident)
                for p in range(2):
                    nc.sync.dma_start(out=wo_sb[p], in_=woT[p * 128:(p + 1) * 128, :])

                for p in range(2):
                    for j in range(NQC):
                        ps_q = pps.tile([128, QC], F32, tag="proj", name="ps_q")
                        for c in range(8):
                            nc.tensor.matmul(
                                ps_q, wq_sb[c][:, p * 128:(p + 1) * 128],
                                xt[c][:, j * QC:(j + 1) * QC],
                                start=(c == 0), stop=(c == 7))
                        nc.vector.tensor_scalar_add(
                            qT_sb[p][:, j * QC:(j + 1) * QC], ps_q,
                            bq_sb[:, p:p + 1])
                        ps_k = pps.tile([128, QC], F32, tag="proj", name="ps_k")
                        for c in range(8):
                            nc.tensor.matmul(
                                ps_k, wk_sb[c][:, p * 128:(p + 1) * 128],
                                xt[c][:, j * QC:(j + 1) * QC],
                                start=(c == 0), stop=(c == 7))
                        nc.vector.tensor_copy(
                            kT_sb[p][:, j * QC:(j + 1) * QC], ps_k)

                for t in range(NKT):
                    ps_v = pps.tile([128, 256], F32, tag="vps", name="ps_v")
                    for c in range(8):
                        nc.tensor.matmul(
                            ps_v, xt[c][:, t * 128:(t + 1) * 128], wv_sb[c],
                            start=(c == 0), stop=(c == 7))
                    v_view = v_sb[t].rearrange("p (h w) -> p h w", w=65)
                    nc.vector.memset(v_view[:, :, 64:65], 1.0)
                    nc.vector.tensor_copy(
                        v_view[:, :, 0:64],
                        ps_v.rearrange("p (h w) -> p h w", w=64))

            if debug:
                for p in range(2):
                    nc.sync.dma_start(out=dbg["qT"][:, p, :], in_=qT_sb[p])
                    nc.sync.dma_start(out=dbg["kT"][:, p, :], in_=kT_sb[p])
                for t in range(NKT):
                    nc.sync.dma_start(out=dbg["vv"][:, t, :], in_=v_sb[t])

            # ---------------- Phase B: attention ----------------
            with (
                tc.tile_pool(name="sq", bufs=2, space="PSUM") as sqp,
                tc.tile_pool(name="ops", bufs=2, space="PSUM") as opp,
                tc.tile_pool(name="ep", bufs=3) as ep,
                tc.tile_pool(name="osb", bufs=3) as osbp,
                tc.tile_pool(name="rp", bufs=3) as rp,
            ):
                deferred_norm = [None]

                def emit_norm(p_, q0_, o_ps_):
                    for s in range(2):
                        o_sb = osbp.tile([64, QC], F32, tag="osb", name="o_sb")
                        nc.vector.tensor_copy(o_sb, o_ps_[s][0:64, :])
                        sums = rp.tile([1, QC], F32, tag="sums", name="sums")
                        nc.vector.tensor_copy(sums, o_ps_[s][64:65, :])
                        recip = rp.tile([1, QC], F32, tag="recip", name="recip")
                        nc.vector.reciprocal_approx_fast(out=recip, in_=sums)
                        bc = rp.tile([64, QC], F32, tag="bc", name="bc")
                        nc.gpsimd.partition_broadcast(bc, recip)
                        nc.vector.tensor_mul(
                            outT_sb[p_][s * 64:(s + 1) * 64, q0_:q0_ + QC],
                            o_sb, bc)

                for p in range(2):
                    for qc in range(NQC):
                        q0 = qc * QC
                        nkt = 4 * qc + 4
                        o_ps = [opp.tile([65, QC], F32, tag=f"o{s}", name=f"ops{s}")
                                for s in range(2)]
                        pend = None
                        for kt in range(nkt):
                            o = kt * 128 - q0
                            diag = o >= 0
                            s_ab = sqp.tile([128, 2 * QC], F32, tag="sq", name="s_ab")
                            lo = o if diag else 0
                            if diag:
                                oi = o // 128
                                for s in range(2):
                                    nc.tensor.matmul(
                                        s_ab[:, s * QC:s * QC + o + 128], id_sb,
                                        mask_sb[:, oi * 512:oi * 512 + o + 128],
                                        start=True, stop=False,
                                        skip_group_check=True)
                            for s in range(2):
                                half = s * QC
                                nc.tensor.matmul(
                                    s_ab[:, half + lo:half + QC],
                                    kT_sb[p][s * 64:(s + 1) * 64,
                                             kt * 128:(kt + 1) * 128],
                                    qT_sb[p][s * 64:(s + 1) * 64,
                                             q0 + lo:q0 + QC],
                                    start=not diag, stop=True,
                                    tile_position=(s * 64, 0),
                                    skip_group_check=True)
                            e_ab = ep.tile([128, 2 * QC], BF16, tag="e", name="e_ab")
                            nc.scalar.activation(e_ab, s_ab, AF.Exp, scale=0.125)
                            if kt == 2 and deferred_norm[0] is not None:
                                deferred_norm[0]()
                                deferred_norm[0] = None
                            if pend is not None:
                                _kt, _e, _lo = pend
                                for s in range(2):
                                    hb = 2 * p + s
                                    nc.tensor.matmul(
                                        o_ps[s][:, _lo:QC],
                                        v_sb[_kt][:, hb * 65:(hb + 1) * 65],
                                        _e[:, s * QC + _lo:(s + 1) * QC],
                                        start=(_kt == 0), stop=False,
                                        skip_group_check=True)
                            pend = (kt, e_ab, o if diag else 0)
                        _kt, _e, _lo = pend
                        for s in range(2):
                            hb = 2 * p + s
                            nc.tensor.matmul(
                                o_ps[s][:, _lo:QC],
                                v_sb[_kt][:, hb * 65:(hb + 1) * 65],
                                _e[:, s * QC + _lo:(s + 1) * QC],
                                start=False, stop=True,
                                skip_group_check=True)
                        deferred_norm[0] = (
                            lambda p_=p, q0_=q0, o_=o_ps: emit_norm(p_, q0_, o_))
                if deferred_norm[0] is not None:
                    deferred_norm[0]()
                    deferred_norm[0] = None

            if debug:
                for p in range(2):
                    nc.sync.dma_start(out=dbg["oT"][:, p, :], in_=outT_sb[p])

            # ---------------- Phase C: output projection ----------------
            with (
                tc.tile_pool(name="fps", bufs=4, space="PSUM") as fps,
                tc.tile_pool(name="fsb", bufs=4) as fsb,
            ):
                for qt in range(16):
                    for oc in range(2):
                        f_ps = fps.tile([128, 512], F32, tag="f", name="f_ps")
                        for p in range(2):
                            nc.tensor.matmul(
                                f_ps, outT_sb[p][:, qt * 128:(qt + 1) * 128],
                                wo_sb[p][:, oc * 512:(oc + 1) * 512],
                                start=(p == 0), stop=(p == 1))
                        f_sb = fsb.tile([128, 512], F32, tag="f", name="f_sb")
                        if (qt * 2 + oc) % 2 == 0:
                            nc.scalar.copy(f_sb, f_ps)
                        else:
                            nc.vector.tensor_copy(f_sb, f_ps)
                        nc.sync.dma_start(
                            out=y[qt * 128:(qt + 1) * 128,
                                  oc * 512:(oc + 1) * 512],
                            in_=f_sb)

    nc.compile()
    return nc


_cached = {}


def _get_nc(debug=False):
    key = bool(debug)
    if key not in _cached:
        _cached[key] = _build(debug)
    return _cached[key]


def _host_masks():
    k = np.arange(128)[:, None]
    m = np.zeros((128, 4, 512), np.float32)
    for oi in range(4):
        o = oi * 128
        q = np.arange(512)[None, :]
        m[:, oi, :] = np.where(q >= k + o, 0.0, NEG_BIG)
    return m.reshape(128, 2048).astype(NPBF)


def _prep_inputs(x, w_q, b_q, w_k, w_v):
    masks = _host_masks()
    ident = np.eye(128, dtype=NPBF)
    wqT_f = np.ascontiguousarray(w_q.T).astype(NPBF)
    wkT_f = np.ascontiguousarray(w_k.T).astype(NPBF)
    wvT_f = np.ascontiguousarray(w_v.T).astype(NPBF)
    in_maps = []
    for core in range(N_CORES):
        b, hg = divmod(core, 4)
        cs = slice(hg * 256, (hg + 1) * 256)
        in_maps.append({
            "xT": np.ascontiguousarray(x[b].T).astype(NPBF),
            "wqT": np.ascontiguousarray(wqT_f[:, cs]),
            "wkT": np.ascontiguousarray(wkT_f[:, cs]),
            "wvT": np.ascontiguousarray(wvT_f[:, cs]),
            "bq2": np.ascontiguousarray(
                b_q[hg * 256:(hg + 1) * 256].reshape(2, 128).T.astype(np.float32)),
            "masks": masks,
            "ident": ident,
        })
    return in_maps


def _numpy_reference(x, attention_mask, w_q, b_q, w_k, b_k, w_v, b_v, w_o, b_o):
    x = x.astype(np.float64)
    q = (x @ w_q.T + b_q).reshape(B, S, H, DK).transpose(0, 2, 1, 3)
    k = (x @ w_k.T + b_k).reshape(B, S, H, DK).transpose(0, 2, 1, 3)
    v = (x @ w_v.T + b_v).reshape(B, S, H, DK).transpose(0, 2, 1, 3)
    scores = np.einsum("bhqd,bhkd->bhqk", q, k) / np.sqrt(DK)
    causal = np.tril(np.ones((S, S), bool))
    mask = causal[None, None] & (attention_mask[:, None, None, :] != 0)
    scores = np.where(mask, scores, -np.inf)
    scores -= scores.max(-1, keepdims=True)
    e = np.exp(scores)
    attn = e / e.sum(-1, keepdims=True)
    out = np.einsum("bhqk,bhkd->bhqd", attn, v)
    out = out.transpose(0, 2, 1, 3).reshape(B, S, D)
    return (out @ w_o.T + b_o).astype(np.float32)


def kernel(x, attention_mask, w_q, b_q, w_k, b_k, w_v, b_v, w_o, b_o,
           _debug=False, _trace=False):
    x = np.asarray(x, np.float32)
    attention_mask = np.asarray(attention_mask)
    if not np.all(attention_mask != 0):
        return _numpy_reference(np.asarray(x), np.asarray(attention_mask),
                                *[np.asarray(a) for a in
                                  (w_q, b_q, w_k, b_k, w_v, b_v, w_o, b_o)])
    w_q, w_k, w_v, w_o = [np.asarray(w, np.float32) for w in (w_q, w_k, w_v, w_o)]
    b_q, b_k, b_v, b_o = [np.asarray(b, np.float32) for b in (b_q, b_k, b_v, b_o)]

    nc = _get_nc(_debug)
    in_maps = _prep_inputs(x, w_q, b_q, w_k, w_v)
    woT_f = np.ascontiguousarray(w_o.T).astype(NPBF)
    for core in range(N_CORES):
        hg = core % 4
        in_maps[core]["woT"] = np.ascontiguousarray(
            woT_f[hg * 256:(hg + 1) * 256, :])

    res = run_bass_kernel_spmd(nc, in_maps, list(range(N_CORES)), trace=_trace)
    const_row = (b_v @ w_o.T + b_o).astype(np.float32)
    y = np.zeros((B, S, D), np.float32)
    for core in range(N_CORES):
        b = core // 4
        y[b] += res.results[core]["y"]
    y += const_row
    if _debug or _trace:
        return y, res
    return y
